# revision 27
# baseline (speedup 1.0000x reference)
"""Trainium2 Bass kernel for nn_AttentionLayer (dense transformer block with
summed heads), distributed over 8 NeuronCores.

Sharding: 4 batches x 2 head-groups (8 heads each). Each core computes
q/k/v projections + causal attention for its 8 heads on its batch in bf16
(fp32 PSUM accumulation), the two cores of a batch exchange the summed head
output via two pipelined pairwise ReduceScatters (each core ends up with
two query-quarters), then each core runs the ff Dense + exact gelu on its
512 queries. The host re-assembles the [4,1024,1024] fp32 output.

Key layout tricks:
- causal mask applied on the TensorEngine: a constant NEG upper-triangle
  stationary x identity moving matmul accumulated into the score PSUM.
- softmax exp runs as merged [128,1024] Activation instructions over
  2-chunk PSUM tiles; invalid (non-causal) columns are exp'd but never
  read by the z matmuls, which only cover the causal trapezoid.
- softmax denominators come from an ones-column augmented V (row 64 of the
  z PSUM), inverted with the fast approximate DVE reciprocal.
- ff bias is folded into the ff matmul via an ones-row augmented stationary.
"""

import sys

sys.path.insert(0, "/opt/trn_rl_repo")

import numpy as np

import concourse.bass as bass
import concourse.bacc as bacc
import concourse.mybir as mybir
import concourse.tile as tile
from concourse.bass_utils import run_bass_kernel_spmd

B, S, D, H, DH = 4, 1024, 1024, 16, 64
HL, NPAIR = 8, 4          # heads / head-pairs per core
NEG = -1.0e11
FP32 = mybir.dt.float32
BF16 = mybir.dt.bfloat16
AF = mybir.ActivationFunctionType
ALU = mybir.AluOpType
RG = [[0, 1], [2, 3], [4, 5], [6, 7]]


def build_nc():
    nc = bacc.Bacc("TRN2", target_bir_lowering=False, num_devices=8)

    xT = nc.declare_dram_parameter("xT", [D, S], BF16, isOutput=False)
    wq = nc.declare_dram_parameter("wq", [D, HL * DH], BF16, isOutput=False)
    wk = nc.declare_dram_parameter("wk", [D, HL * DH], BF16, isOutput=False)
    wv = nc.declare_dram_parameter("wv", [D, HL * DH], BF16, isOutput=False)
    bqk = nc.declare_dram_parameter("bqk", [128, 2 * NPAIR], FP32, isOutput=False)
    bvt = nc.declare_dram_parameter("bvt", [128, HL * DH], FP32, isOutput=False)
    wfa = nc.declare_dram_parameter("wfa", [DH + 1, D], BF16, isOutput=False)
    trim = nc.declare_dram_parameter("trim", [128, 128], BF16, isOutput=False)
    iden = nc.declare_dram_parameter("iden", [128, 128], BF16, isOutput=False)
    onesp = nc.declare_dram_parameter("onesp", [128, 8, HL], BF16, isOutput=False)
    ones512 = nc.declare_dram_parameter("ones512", [1, 512], BF16, isOutput=False)
    out_ext = nc.declare_dram_parameter("out", [S // 2, D], FP32, isOutput=True)

    with tile.TileContext(nc) as tc:
        with (
            tc.tile_pool(name="const", bufs=1) as constp,
            tc.tile_pool(name="qkv", bufs=1) as qkvp,
            tc.tile_pool(name="dram", bufs=2, space="DRAM") as dramp,
            tc.tile_pool(name="big", bufs=2, space="PSUM") as bigp,
            tc.tile_pool(name="zz", bufs=4, space="PSUM") as zzp,
            tc.tile_pool(name="small", bufs=3) as smallp,
            tc.tile_pool(name="outp", bufs=3) as outp,
            tc.tile_pool(name="xp", bufs=8) as xp,
            tc.tile_pool(name="wtp", bufs=3) as wtp,
            tc.tile_pool(name="wpool", bufs=2) as wpool,
            nc.allow_low_precision(reason="bf16 activations, fp32 accumulation"),
        ):
            # ---- inputs first: big weight DMAs, per-chunk x DMAs ----
            xts = [xp.tile([128, S], BF16, tag="xT", name=f"xts{i}")
                   for i in range(8)]
            wq_sb = wtp.tile([128, 8, HL * DH], BF16, tag="w")
            wk_sb = wtp.tile([128, 8, HL * DH], BF16, tag="w")
            wv_sb = wtp.tile([128, 8, HL * DH], BF16, tag="w")
            for dc in range(8):
                nc.sync.dma_start(xts[dc][:], xT[dc * 128:(dc + 1) * 128, :])
                nc.sync.dma_start(wq_sb[:, dc, :], wq[dc * 128:(dc + 1) * 128, :])
                nc.sync.dma_start(wk_sb[:, dc, :], wk[dc * 128:(dc + 1) * 128, :])
                nc.sync.dma_start(wv_sb[:, dc, :], wv[dc * 128:(dc + 1) * 128, :])

            # ---- constants ----
            bqk_sb = constp.tile([128, 2 * NPAIR], FP32)
            nc.sync.dma_start(bqk_sb[:], bqk[:])
            bvt_sb = constp.tile([128, HL * DH], FP32)
            nc.sync.dma_start(bvt_sb[:], bvt[:])
            tri_sb = constp.tile([128, 128], BF16)
            nc.sync.dma_start(tri_sb[:], trim[:])
            id_sb = constp.tile([128, 128], BF16)
            nc.sync.dma_start(id_sb[:], iden[:])
            wfa_sb = constp.tile([DH + 1, D], BF16)
            nc.sync.dma_start(wfa_sb[:], wfa[:])

            # ---- persistent activations ----
            qT = qkvp.tile([128, NPAIR, S], BF16)   # [(2 heads)*64e, pair, s]
            kT = qkvp.tile([128, NPAIR, S], BF16)
            vA = qkvp.tile([128, 8, HL, DH + 1], BF16)  # [t_in, t_chunk, head, e+1]
            zsum = qkvp.tile([DH, S], FP32)
            zsumB = qkvp.tile([DH, 512], FP32)  # sc=1 heads 4-7 partial sum
            rsA = qkvp.tile([DH, 256], FP32)    # received quarter, sc=0
            rsB1 = qkvp.tile([DH, 256], FP32)   # received quarter, sc=1 h0-3
            rsB2 = qkvp.tile([DH, 256], FP32)   # received quarter, sc=1 h4-7
            rs_aug = qkvp.tile([DH + 1, 512], BF16)
            nc.sync.dma_start(vA[:, :, :, DH:DH + 1], onesp[:])
            nc.sync.dma_start(rs_aug[DH:DH + 1, :], ones512[:])

            # ---- phase 1: projections (bf16 matmuls, fp32 psum) ----
            for p in range(NPAIR):
                psq = bigp.tile([128, 1024], FP32, tag="big")
                for s2 in range(2):
                    ssl = slice(s2 * 512, (s2 + 1) * 512)
                    for dc in range(8):
                        nc.tensor.matmul(
                            psq[:, ssl],
                            wq_sb[:, dc, p * 128:(p + 1) * 128],
                            xts[dc][:, ssl],
                            start=(dc == 0), stop=(dc == 7),
                        )
                nc.vector.tensor_scalar_add(
                    qT[:, p, :], psq[:], bqk_sb[:, p:p + 1])
                psk = bigp.tile([128, 1024], FP32, tag="big")
                for s2 in range(2):
                    ssl = slice(s2 * 512, (s2 + 1) * 512)
                    for dc in range(8):
                        nc.tensor.matmul(
                            psk[:, ssl],
                            wk_sb[:, dc, p * 128:(p + 1) * 128],
                            xts[dc][:, ssl],
                            start=(dc == 0), stop=(dc == 7),
                        )
                nc.vector.tensor_scalar_add(
                    kT[:, p, :], psk[:], bqk_sb[:, NPAIR + p:NPAIR + p + 1])

            for u in range(4):
                psv = bigp.tile([128, 1024], FP32, tag="big")
                for half in range(2):
                    t_c = 2 * u + half
                    for dc in range(8):
                        nc.tensor.matmul(
                            psv[:, half * 512:(half + 1) * 512],
                            xts[dc][:, t_c * 128:(t_c + 1) * 128],
                            wv_sb[:, dc, :],
                            start=(dc == 0), stop=(dc == 7),
                        )
                for half in range(2):
                    t_c = 2 * u + half
                    nc.vector.tensor_tensor(
                        vA[:, t_c, :, :DH],
                        psv[:, half * 512:(half + 1) * 512].rearrange(
                            "p (h e) -> p h e", h=HL),
                        bvt_sb[:].rearrange("p (h e) -> p h e", h=HL),
                        ALU.add,
                    )

            # ---- phase 2: attention, query-half (sc) outer for RS overlap ----
            def ff_block(j0, j1):
                for j in range(j0, j1):
                    for dcol in range(2):
                        dsl = slice(dcol * 512, (dcol + 1) * 512)
                        fps = zzp.tile([128, 512], FP32, tag="zz")
                        nc.tensor.matmul(
                            fps[:],
                            rs_aug[:, j * 128:(j + 1) * 128],
                            wfa_sb[:, dsl],
                            start=True, stop=True,
                        )
                        og = outp.tile([128, 512], FP32, tag="og")
                        nc.scalar.activation(og[:], fps[:], AF.Gelu)
                        nc.sync.dma_start(
                            out_ext[j * 128:(j + 1) * 128, dsl], og[:])

            def exchange(src_ap, rs_f32_tile):
                """Pairwise quarter ReduceScatter of a [64,512] fp32 half;
                rank g of the pair receives quarter g into rs_f32_tile."""
                zin = dramp.tile([2, DH, 256], FP32)
                zout = dramp.tile([DH, 256], FP32)
                for seg in range(2):
                    nc.sync.dma_start(
                        zin[seg], src_ap[:, seg * 256:(seg + 1) * 256])
                nc.gpsimd.collective_compute(
                    "ReduceScatter", ALU.add, replica_groups=RG,
                    ins=[zin[:].opt()], outs=[zout[:].opt()],
                )
                nc.sync.dma_start(rs_f32_tile[:], zout[:])

            for sc in range(2):
                C = 4 * sc + 4
                qsl = slice(sc * 512, (sc + 1) * 512)
                for h_loc in range(HL):
                    p, hh = h_loc // 2, h_loc % 2
                    rows = slice(hh * 64, hh * 64 + 64)
                    # sc=1 heads 4-7 accumulate separately so their exchange
                    # can start after the first four heads
                    acc = zsum[:, qsl] if (sc == 0 or h_loc < 4) else zsumB[:]
                    first = h_loc == 0 or (sc == 1 and h_loc == 4)
                    wT = wpool.tile([128, 8, 512], BF16, tag="wT")
                    for u in range(C // 2):
                        ps = bigp.tile([128, 1024], FP32, tag="big")
                        for half in range(2):
                            t_c = 2 * u + half
                            csl = slice(half * 512, (half + 1) * 512)
                            diag = t_c >= 4 * sc
                            nc.tensor.matmul(
                                ps[:, csl],
                                kT[rows, p, t_c * 128:(t_c + 1) * 128],
                                qT[rows, p, qsl],
                                start=True, stop=not diag,
                                skip_group_check=True,
                            )
                            if diag:
                                off = half * 512 + 128 * (t_c - 4 * sc)
                                nc.tensor.matmul(
                                    ps[:, off:off + 128],
                                    tri_sb[:],
                                    id_sb[:],
                                    start=False, stop=True,
                                    skip_group_check=True,
                                )
                        nc.scalar.activation(
                            wT[:, 2 * u:2 * u + 2, :], ps[:], AF.Exp, scale=0.125)
                    zaug = zzp.tile([128, 512], FP32, tag="zz")
                    for t_c in range(C):
                        zs = 128 * (t_c - 4 * sc) if t_c >= 4 * sc else 0
                        nc.tensor.matmul(
                            zaug[:DH + 1, zs:512],
                            vA[:, t_c, h_loc, :],
                            wT[:, t_c, zs:512],
                            start=(t_c == 0), stop=(t_c == C - 1),
                            skip_group_check=True,
                        )
                    # custom-DVE reciprocal requires base partition 0: copy
                    # the den row (psum partition 64) down to an SBUF tile.
                    dcopy = smallp.tile([1, 512], FP32, tag="dcopy")
                    nc.vector.tensor_copy(dcopy[:], zaug[DH:DH + 1, :])
                    recip = smallp.tile([1, 512], FP32, tag="recip")
                    nc.vector.reciprocal_approx_fast(recip[:], dcopy[:])
                    bcast = smallp.tile([DH, 512], FP32, tag="bcast")
                    nc.gpsimd.partition_broadcast(bcast[:], recip[:])
                    if first:
                        nc.vector.tensor_tensor(
                            acc, zaug[:DH, :], bcast[:], ALU.mult)
                    else:
                        tmp = smallp.tile([DH, 512], FP32, tag="ztmp")
                        nc.vector.tensor_tensor(
                            tmp[:], zaug[:DH, :], bcast[:], ALU.mult)
                        nc.vector.tensor_tensor(acc, acc, tmp[:], ALU.add)

                    if sc == 1 and h_loc == 3:
                        # exchange heads 0-3 of the second half early
                        exchange(zsum[:, qsl], rsB1)
                    if sc == 1 and h_loc == 5:
                        # first-quarter ff hidden under the sc=1 head loop,
                        # once the sc=0 exchange has certainly landed
                        nc.vector.tensor_copy(rs_aug[:DH, 0:256], rsA[:])
                        ff_block(0, 2)

                if sc == 0:
                    exchange(zsum[:, qsl], rsA)

            # heads 4-7 of the second half: final, smallest exposed exchange
            exchange(zsumB[:], rsB2)
            nc.vector.tensor_tensor(rsB1[:], rsB1[:], rsB2[:], ALU.add)
            nc.vector.tensor_copy(rs_aug[:DH, 256:512], rsB1[:])

            # ---- phase 3: remaining ff quarter (queries from sc=1) ----
            ff_block(2, 4)

    nc.compile()
    return nc


_NC = None


def _get_nc():
    global _NC
    if _NC is None:
        _NC = build_nc()
    return _NC


def make_in_maps(x, Wq, bq, Wk, bk, Wv, bv, Wf, bf):
    import ml_dtypes

    bf16 = ml_dtypes.bfloat16
    x, Wq, bq, Wk, bk, Wv, bv, Wf, bf = (
        np.asarray(a, dtype=np.float32)
        for a in (x, Wq, bq, Wk, bk, Wv, bv, Wf, bf))

    ii, jj = np.meshgrid(np.arange(128), np.arange(128), indexing="ij")
    # stationary triangle for the mask matmul: rows k (contraction), cols m;
    # NEG where m > k so that (tri^T @ I)[m, j] = NEG * (m > j)
    trim = np.where(jj > ii, np.float32(NEG), 0.0).astype(bf16)
    iden = np.eye(128, dtype=np.float32).astype(bf16)
    wfa = np.concatenate([Wf, bf.reshape(1, D)], axis=0).astype(bf16)

    in_maps = []
    for c in range(8):
        b, g = c // 2, c % 2
        hs = slice(g * HL, (g + 1) * HL)
        bqk_l = np.empty((128, 2 * NPAIR), np.float32)
        for p in range(NPAIR):
            bqk_l[:, p] = bq[g * HL + 2 * p: g * HL + 2 * p + 2].reshape(128)
            bqk_l[:, NPAIR + p] = bk[g * HL + 2 * p: g * HL + 2 * p + 2].reshape(128)
        in_maps.append({
            "xT": np.ascontiguousarray(x[b].T).astype(bf16),
            "wq": np.ascontiguousarray(
                Wq[hs].transpose(1, 0, 2).reshape(D, HL * DH)).astype(bf16),
            "wk": np.ascontiguousarray(
                Wk[hs].transpose(1, 0, 2).reshape(D, HL * DH)).astype(bf16),
            "wv": np.ascontiguousarray(
                Wv[hs].transpose(1, 0, 2).reshape(D, HL * DH)).astype(bf16),
            "bqk": bqk_l,
            "bvt": np.ascontiguousarray(
                np.broadcast_to(bv[hs].reshape(1, HL * DH), (128, HL * DH))),
            "wfa": wfa,
            "trim": trim,
            "iden": iden,
            "onesp": np.ones((128, 8, HL), bf16),
            "ones512": np.ones((1, 512), bf16),
        })
    return in_maps


def run(in_maps, trace=False, **kw):
    nc = _get_nc()
    return run_bass_kernel_spmd(nc, in_maps, list(range(8)), trace=trace, **kw)


def assemble(results):
    """Reassemble per-core [512, D] outputs into [B, S, D].

    Core c = (b, g): rows 0-255 are queries [256g, 256g+256), rows 256-511
    are queries [512+256g, 512+256g+256) of batch b.
    """
    out = np.empty((B, S, D), np.float32)
    for c in range(8):
        b, g = c // 2, c % 2
        r = results[c]["out"]
        out[b, 256 * g:256 * (g + 1), :] = r[0:256]
        out[b, 512 + 256 * g:512 + 256 * (g + 1), :] = r[256:512]
    return out


def kernel(x, Wq, bq, Wk, bk, Wv, bv, Wf, bf):
    in_maps = make_in_maps(x, Wq, bq, Wk, bk, Wv, bv, Wf, bf)
    res = run(in_maps)
    return assemble(res.results)


if __name__ == "__main__":
    nc = build_nc()
    print("build OK")


# revision 28
# speedup vs baseline: 1.0951x; 1.0951x over previous
"""Trainium2 Bass kernel for nn_AttentionLayer (dense transformer block with
summed heads), distributed over 8 NeuronCores.

Sharding: 4 batches x 2 head-groups (8 heads each). Each core computes
q/k/v projections + causal attention for its 8 heads on its batch in bf16
(fp32 PSUM accumulation), the two cores of a batch exchange the summed head
output via two pipelined pairwise ReduceScatters (each core ends up with
two query-quarters), then each core runs the ff Dense + exact gelu on its
512 queries. The host re-assembles the [4,1024,1024] fp32 output.

Key layout tricks:
- causal mask applied on the TensorEngine: a constant NEG upper-triangle
  stationary x identity moving matmul accumulated into the score PSUM.
- softmax exp runs as merged [128,1024] Activation instructions over
  2-chunk PSUM tiles; invalid (non-causal) columns are exp'd but never
  read by the z matmuls, which only cover the causal trapezoid.
- softmax denominators come from an ones-column augmented V (row 64 of the
  z PSUM), inverted with the fast approximate DVE reciprocal.
- ff bias is folded into the ff matmul via an ones-row augmented stationary.
"""

import sys

sys.path.insert(0, "/opt/trn_rl_repo")

import numpy as np

import concourse.bass as bass
import concourse.bacc as bacc
import concourse.mybir as mybir
import concourse.tile as tile
from concourse.bass_utils import run_bass_kernel_spmd

B, S, D, H, DH = 4, 1024, 1024, 16, 64
HL, NPAIR = 8, 4          # heads / head-pairs per core
NEG = -1.0e11
FP32 = mybir.dt.float32
BF16 = mybir.dt.bfloat16
AF = mybir.ActivationFunctionType
ALU = mybir.AluOpType
RG = [[0, 1], [2, 3], [4, 5], [6, 7]]


def build_nc():
    nc = bacc.Bacc("TRN2", target_bir_lowering=False, num_devices=8)

    xT = nc.declare_dram_parameter("xT", [D, S], BF16, isOutput=False)
    wq = nc.declare_dram_parameter("wq", [D, HL * DH], BF16, isOutput=False)
    wk = nc.declare_dram_parameter("wk", [D, HL * DH], BF16, isOutput=False)
    wv = nc.declare_dram_parameter("wv", [D, HL * DH], BF16, isOutput=False)
    bqk = nc.declare_dram_parameter("bqk", [128, 2 * NPAIR], FP32, isOutput=False)
    bvt = nc.declare_dram_parameter("bvt", [128, HL * DH], FP32, isOutput=False)
    wfa = nc.declare_dram_parameter("wfa", [DH + 1, D], BF16, isOutput=False)
    trim = nc.declare_dram_parameter("trim", [128, 128], BF16, isOutput=False)
    iden = nc.declare_dram_parameter("iden", [128, 128], BF16, isOutput=False)
    onesp = nc.declare_dram_parameter("onesp", [128, 8, HL], BF16, isOutput=False)
    ones512 = nc.declare_dram_parameter("ones512", [1, 512], BF16, isOutput=False)
    out_ext = nc.declare_dram_parameter("out", [S // 2, D], FP32, isOutput=True)

    with tile.TileContext(nc) as tc:
        with (
            tc.tile_pool(name="const", bufs=1) as constp,
            tc.tile_pool(name="qkv", bufs=1) as qkvp,
            tc.tile_pool(name="dram", bufs=2, space="DRAM") as dramp,
            tc.tile_pool(name="big", bufs=2, space="PSUM") as bigp,
            tc.tile_pool(name="zz", bufs=4, space="PSUM") as zzp,
            tc.tile_pool(name="small", bufs=3) as smallp,
            tc.tile_pool(name="outp", bufs=3) as outp,
            tc.tile_pool(name="xp", bufs=8) as xp,
            tc.tile_pool(name="wtp", bufs=3) as wtp,
            tc.tile_pool(name="wpool", bufs=2) as wpool,
            nc.allow_low_precision(reason="bf16 activations, fp32 accumulation"),
        ):
            # ---- inputs first: big weight DMAs, per-chunk x DMAs ----
            xts = [xp.tile([128, S], BF16, tag="xT", name=f"xts{i}")
                   for i in range(8)]
            wq_sb = wtp.tile([128, 8, HL * DH], BF16, tag="w")
            wk_sb = wtp.tile([128, 8, HL * DH], BF16, tag="w")
            wv_sb = wtp.tile([128, 8, HL * DH], BF16, tag="w")
            nc.sync.dma_start(
                wq_sb[:], wq.rearrange("(dc p) c -> p dc c", p=128))
            for dc in range(4):
                nc.sync.dma_start(xts[dc][:], xT[dc * 128:(dc + 1) * 128, :])
            nc.sync.dma_start(
                wk_sb[:], wk.rearrange("(dc p) c -> p dc c", p=128))
            for dc in range(4, 8):
                nc.sync.dma_start(xts[dc][:], xT[dc * 128:(dc + 1) * 128, :])
            nc.sync.dma_start(
                wv_sb[:], wv.rearrange("(dc p) c -> p dc c", p=128))

            # ---- constants ----
            bqk_sb = constp.tile([128, 2 * NPAIR], FP32)
            nc.sync.dma_start(bqk_sb[:], bqk[:])
            bvt_sb = constp.tile([128, HL * DH], FP32)
            nc.sync.dma_start(bvt_sb[:], bvt[:])
            tri_sb = constp.tile([128, 128], BF16)
            nc.sync.dma_start(tri_sb[:], trim[:])
            id_sb = constp.tile([128, 128], BF16)
            nc.sync.dma_start(id_sb[:], iden[:])
            wfa_sb = constp.tile([DH + 1, D], BF16)
            nc.sync.dma_start(wfa_sb[:], wfa[:])

            # ---- persistent activations ----
            qT = qkvp.tile([128, NPAIR, S], BF16)   # [(2 heads)*64e, pair, s]
            kT = qkvp.tile([128, NPAIR, S], BF16)
            vA = qkvp.tile([128, 8, HL, DH + 1], BF16)  # [t_in, t_chunk, head, e+1]
            zsum = qkvp.tile([DH, S], FP32)
            zsumB = qkvp.tile([DH, 512], FP32)  # sc=1 heads 4-7 partial sum
            rsA = qkvp.tile([DH, 256], FP32)    # received quarter, sc=0
            rsB1 = qkvp.tile([DH, 256], FP32)   # received quarter, sc=1 h0-3
            rsB2 = qkvp.tile([DH, 256], FP32)   # received quarter, sc=1 h4-7
            rs_aug = qkvp.tile([DH + 1, 512], BF16)
            nc.sync.dma_start(vA[:, :, :, DH:DH + 1], onesp[:])
            nc.sync.dma_start(rs_aug[DH:DH + 1, :], ones512[:])

            # ---- phase 1: projections (bf16 matmuls, fp32 psum) ----
            for p in range(NPAIR):
                psq = bigp.tile([128, 1024], FP32, tag="big")
                for s2 in range(2):
                    ssl = slice(s2 * 512, (s2 + 1) * 512)
                    for dc in range(8):
                        nc.tensor.matmul(
                            psq[:, ssl],
                            wq_sb[:, dc, p * 128:(p + 1) * 128],
                            xts[dc][:, ssl],
                            start=(dc == 0), stop=(dc == 7),
                        )
                nc.vector.tensor_scalar_add(
                    qT[:, p, :], psq[:], bqk_sb[:, p:p + 1])
                psk = bigp.tile([128, 1024], FP32, tag="big")
                for s2 in range(2):
                    ssl = slice(s2 * 512, (s2 + 1) * 512)
                    for dc in range(8):
                        nc.tensor.matmul(
                            psk[:, ssl],
                            wk_sb[:, dc, p * 128:(p + 1) * 128],
                            xts[dc][:, ssl],
                            start=(dc == 0), stop=(dc == 7),
                        )
                nc.vector.tensor_scalar_add(
                    kT[:, p, :], psk[:], bqk_sb[:, NPAIR + p:NPAIR + p + 1])

            for u in range(4):
                psv = bigp.tile([128, 1024], FP32, tag="big")
                for half in range(2):
                    t_c = 2 * u + half
                    for dc in range(8):
                        nc.tensor.matmul(
                            psv[:, half * 512:(half + 1) * 512],
                            xts[dc][:, t_c * 128:(t_c + 1) * 128],
                            wv_sb[:, dc, :],
                            start=(dc == 0), stop=(dc == 7),
                        )
                for half in range(2):
                    t_c = 2 * u + half
                    nc.vector.tensor_tensor(
                        vA[:, t_c, :, :DH],
                        psv[:, half * 512:(half + 1) * 512].rearrange(
                            "p (h e) -> p h e", h=HL),
                        bvt_sb[:].rearrange("p (h e) -> p h e", h=HL),
                        ALU.add,
                    )

            # ---- phase 2: attention, query-half (sc) outer for RS overlap ----
            def ff_block(j0, j1):
                for j in range(j0, j1):
                    for dcol in range(2):
                        dsl = slice(dcol * 512, (dcol + 1) * 512)
                        fps = zzp.tile([128, 512], FP32, tag="zz")
                        nc.tensor.matmul(
                            fps[:],
                            rs_aug[:, j * 128:(j + 1) * 128],
                            wfa_sb[:, dsl],
                            start=True, stop=True,
                        )
                        og = outp.tile([128, 512], FP32, tag="og")
                        nc.scalar.activation(og[:], fps[:], AF.Gelu)
                        nc.sync.dma_start(
                            out_ext[j * 128:(j + 1) * 128, dsl], og[:])

            def exchange(src_ap, rs_f32_tile):
                """Pairwise quarter ReduceScatter of a [64,512] fp32 half;
                rank g of the pair receives quarter g into rs_f32_tile."""
                zin = dramp.tile([2, DH, 256], FP32)
                zout = dramp.tile([DH, 256], FP32)
                for seg in range(2):
                    nc.sync.dma_start(
                        zin[seg], src_ap[:, seg * 256:(seg + 1) * 256])
                nc.gpsimd.collective_compute(
                    "ReduceScatter", ALU.add, replica_groups=RG,
                    ins=[zin[:].opt()], outs=[zout[:].opt()],
                )
                nc.sync.dma_start(rs_f32_tile[:], zout[:])

            for sc in range(2):
                C = 4 * sc + 4
                qsl = slice(sc * 512, (sc + 1) * 512)
                for h_loc in range(HL):
                    p, hh = h_loc // 2, h_loc % 2
                    rows = slice(hh * 64, hh * 64 + 64)
                    # sc=1 heads 4-7 accumulate separately so their exchange
                    # can start after the first four heads
                    acc = zsum[:, qsl] if (sc == 0 or h_loc < 4) else zsumB[:]
                    first = h_loc == 0 or (sc == 1 and h_loc == 4)
                    wT = wpool.tile([128, 8, 512], BF16, tag="wT")
                    for u in range(C // 2):
                        ps = bigp.tile([128, 1024], FP32, tag="big")
                        for half in range(2):
                            t_c = 2 * u + half
                            diag = t_c >= 4 * sc
                            zs = 128 * (t_c - 4 * sc) if diag else 0
                            off = half * 512 + zs
                            nc.tensor.matmul(
                                ps[:, off:(half + 1) * 512],
                                kT[rows, p, t_c * 128:(t_c + 1) * 128],
                                qT[rows, p, sc * 512 + zs:(sc + 1) * 512],
                                start=True, stop=not diag,
                                skip_group_check=True,
                            )
                            if diag:
                                nc.tensor.matmul(
                                    ps[:, off:off + 128],
                                    tri_sb[:],
                                    id_sb[:],
                                    start=False, stop=True,
                                    skip_group_check=True,
                                )
                        nc.scalar.activation(
                            wT[:, 2 * u:2 * u + 2, :], ps[:], AF.Exp, scale=0.125)
                    zaug = zzp.tile([128, 512], FP32, tag="zz")
                    for t_c in range(C):
                        zs = 128 * (t_c - 4 * sc) if t_c >= 4 * sc else 0
                        nc.tensor.matmul(
                            zaug[:DH + 1, zs:512],
                            vA[:, t_c, h_loc, :],
                            wT[:, t_c, zs:512],
                            start=(t_c == 0), stop=(t_c == C - 1),
                            skip_group_check=True,
                        )
                    # custom-DVE reciprocal requires base partition 0: copy
                    # the den row (psum partition 64) down to an SBUF tile.
                    dcopy = smallp.tile([1, 512], FP32, tag="dcopy")
                    nc.vector.tensor_copy(dcopy[:], zaug[DH:DH + 1, :])
                    recip = smallp.tile([1, 512], FP32, tag="recip")
                    nc.vector.reciprocal_approx_fast(recip[:], dcopy[:])
                    bcast = smallp.tile([DH, 512], FP32, tag="bcast")
                    nc.gpsimd.partition_broadcast(bcast[:], recip[:])
                    if first:
                        nc.vector.tensor_tensor(
                            acc, zaug[:DH, :], bcast[:], ALU.mult)
                    else:
                        tmp = smallp.tile([DH, 512], FP32, tag="ztmp")
                        nc.vector.tensor_tensor(
                            tmp[:], zaug[:DH, :], bcast[:], ALU.mult)
                        nc.vector.tensor_tensor(acc, acc, tmp[:], ALU.add)

                    if sc == 1 and h_loc == 3:
                        # exchange heads 0-3 of the second half early
                        exchange(zsum[:, qsl], rsB1)
                    if sc == 1 and h_loc == 5:
                        # first-quarter ff hidden under the sc=1 head loop,
                        # once the sc=0 exchange has certainly landed
                        nc.vector.tensor_copy(rs_aug[:DH, 0:256], rsA[:])
                        ff_block(0, 2)

                if sc == 0:
                    exchange(zsum[:, qsl], rsA)

            # heads 4-7 of the second half: final, smallest exposed exchange
            exchange(zsumB[:], rsB2)
            nc.vector.tensor_tensor(rsB1[:], rsB1[:], rsB2[:], ALU.add)
            nc.vector.tensor_copy(rs_aug[:DH, 256:512], rsB1[:])

            # ---- phase 3: remaining ff quarter (queries from sc=1) ----
            ff_block(2, 4)

    nc.compile()
    return nc


_NC = None


def _get_nc():
    global _NC
    if _NC is None:
        _NC = build_nc()
    return _NC


def make_in_maps(x, Wq, bq, Wk, bk, Wv, bv, Wf, bf):
    import ml_dtypes

    bf16 = ml_dtypes.bfloat16
    x, Wq, bq, Wk, bk, Wv, bv, Wf, bf = (
        np.asarray(a, dtype=np.float32)
        for a in (x, Wq, bq, Wk, bk, Wv, bv, Wf, bf))

    ii, jj = np.meshgrid(np.arange(128), np.arange(128), indexing="ij")
    # stationary triangle for the mask matmul: rows k (contraction), cols m;
    # NEG where m > k so that (tri^T @ I)[m, j] = NEG * (m > j)
    trim = np.where(jj > ii, np.float32(NEG), 0.0).astype(bf16)
    iden = np.eye(128, dtype=np.float32).astype(bf16)
    wfa = np.concatenate([Wf, bf.reshape(1, D)], axis=0).astype(bf16)

    in_maps = []
    for c in range(8):
        b, g = c // 2, c % 2
        hs = slice(g * HL, (g + 1) * HL)
        bqk_l = np.empty((128, 2 * NPAIR), np.float32)
        for p in range(NPAIR):
            bqk_l[:, p] = bq[g * HL + 2 * p: g * HL + 2 * p + 2].reshape(128)
            bqk_l[:, NPAIR + p] = bk[g * HL + 2 * p: g * HL + 2 * p + 2].reshape(128)
        in_maps.append({
            "xT": np.ascontiguousarray(x[b].T).astype(bf16),
            "wq": np.ascontiguousarray(
                Wq[hs].transpose(1, 0, 2).reshape(D, HL * DH)).astype(bf16),
            "wk": np.ascontiguousarray(
                Wk[hs].transpose(1, 0, 2).reshape(D, HL * DH)).astype(bf16),
            "wv": np.ascontiguousarray(
                Wv[hs].transpose(1, 0, 2).reshape(D, HL * DH)).astype(bf16),
            "bqk": bqk_l,
            "bvt": np.ascontiguousarray(
                np.broadcast_to(bv[hs].reshape(1, HL * DH), (128, HL * DH))),
            "wfa": wfa,
            "trim": trim,
            "iden": iden,
            "onesp": np.ones((128, 8, HL), bf16),
            "ones512": np.ones((1, 512), bf16),
        })
    return in_maps


def run(in_maps, trace=False, **kw):
    nc = _get_nc()
    return run_bass_kernel_spmd(nc, in_maps, list(range(8)), trace=trace, **kw)


def assemble(results):
    """Reassemble per-core [512, D] outputs into [B, S, D].

    Core c = (b, g): rows 0-255 are queries [256g, 256g+256), rows 256-511
    are queries [512+256g, 512+256g+256) of batch b.
    """
    out = np.empty((B, S, D), np.float32)
    for c in range(8):
        b, g = c // 2, c % 2
        r = results[c]["out"]
        out[b, 256 * g:256 * (g + 1), :] = r[0:256]
        out[b, 512 + 256 * g:512 + 256 * (g + 1), :] = r[256:512]
    return out


def kernel(x, Wq, bq, Wk, bk, Wv, bv, Wf, bf):
    in_maps = make_in_maps(x, Wq, bq, Wk, bk, Wv, bv, Wf, bf)
    res = run(in_maps)
    return assemble(res.results)


if __name__ == "__main__":
    nc = build_nc()
    print("build OK")


# revision 29
# speedup vs baseline: 1.1123x; 1.0157x over previous
"""Trainium2 Bass kernel for nn_AttentionLayer (dense transformer block with
summed heads), distributed over 8 NeuronCores.

Sharding: 4 batches x 2 head-groups (8 heads each). Each core computes
q/k/v projections + causal attention for its 8 heads on its batch in bf16
(fp32 PSUM accumulation), the two cores of a batch exchange the summed head
output via two pipelined pairwise ReduceScatters (each core ends up with
two query-quarters), then each core runs the ff Dense + exact gelu on its
512 queries. The host re-assembles the [4,1024,1024] fp32 output.

Key layout tricks:
- causal mask applied on the TensorEngine: a constant NEG upper-triangle
  stationary x identity moving matmul accumulated into the score PSUM.
- softmax exp runs as merged [128,1024] Activation instructions over
  2-chunk PSUM tiles; invalid (non-causal) columns are exp'd but never
  read by the z matmuls, which only cover the causal trapezoid.
- softmax denominators come from an ones-column augmented V (row 64 of the
  z PSUM), inverted with the fast approximate DVE reciprocal.
- ff bias is folded into the ff matmul via an ones-row augmented stationary.
"""

import sys

sys.path.insert(0, "/opt/trn_rl_repo")

import numpy as np

import concourse.bass as bass
import concourse.bacc as bacc
import concourse.mybir as mybir
import concourse.tile as tile
from concourse.bass_utils import run_bass_kernel_spmd

B, S, D, H, DH = 4, 1024, 1024, 16, 64
HL, NPAIR = 8, 4          # heads / head-pairs per core
NEG = -1.0e11
FP32 = mybir.dt.float32
BF16 = mybir.dt.bfloat16
AF = mybir.ActivationFunctionType
ALU = mybir.AluOpType
RG = [[0, 1], [2, 3], [4, 5], [6, 7]]


def build_nc():
    nc = bacc.Bacc("TRN2", target_bir_lowering=False, num_devices=8)

    xT = nc.declare_dram_parameter("xT", [D, S], BF16, isOutput=False)
    wq = nc.declare_dram_parameter("wq", [D, HL * DH], BF16, isOutput=False)
    wk = nc.declare_dram_parameter("wk", [D, HL * DH], BF16, isOutput=False)
    wv = nc.declare_dram_parameter("wv", [D, HL * DH], BF16, isOutput=False)
    bqk = nc.declare_dram_parameter("bqk", [128, 2 * NPAIR], FP32, isOutput=False)
    bvt = nc.declare_dram_parameter("bvt", [128, HL * DH], FP32, isOutput=False)
    wfa = nc.declare_dram_parameter("wfa", [DH + 1, D], BF16, isOutput=False)
    trim = nc.declare_dram_parameter("trim", [128, 128], BF16, isOutput=False)
    iden = nc.declare_dram_parameter("iden", [128, 128], BF16, isOutput=False)
    onesp = nc.declare_dram_parameter("onesp", [128, 8, HL], BF16, isOutput=False)
    ones512 = nc.declare_dram_parameter("ones512", [1, 512], BF16, isOutput=False)
    out_ext = nc.declare_dram_parameter("out", [S // 2, D], FP32, isOutput=True)

    with tile.TileContext(nc) as tc:
        with (
            tc.tile_pool(name="const", bufs=1) as constp,
            tc.tile_pool(name="qkv", bufs=1) as qkvp,
            tc.tile_pool(name="dram", bufs=2, space="DRAM") as dramp,
            tc.tile_pool(name="big", bufs=3, space="PSUM") as bigp,
            tc.tile_pool(name="zz", bufs=2, space="PSUM") as zzp,
            tc.tile_pool(name="small", bufs=3) as smallp,
            tc.tile_pool(name="outp", bufs=3) as outp,
            tc.tile_pool(name="xp", bufs=8) as xp,
            tc.tile_pool(name="wtp", bufs=3) as wtp,
            tc.tile_pool(name="wpool", bufs=2) as wpool,
            nc.allow_low_precision(reason="bf16 activations, fp32 accumulation"),
        ):
            # ---- inputs first: big weight DMAs, per-chunk x DMAs ----
            xts = [xp.tile([128, S], BF16, tag="xT", name=f"xts{i}")
                   for i in range(8)]
            wq_sb = wtp.tile([128, 8, HL * DH], BF16, tag="w")
            wk_sb = wtp.tile([128, 8, HL * DH], BF16, tag="w")
            wv_sb = wtp.tile([128, 8, HL * DH], BF16, tag="w")
            nc.sync.dma_start(
                wq_sb[:], wq.rearrange("(dc p) c -> p dc c", p=128))
            for dc in range(4):
                nc.sync.dma_start(xts[dc][:], xT[dc * 128:(dc + 1) * 128, :])
            nc.sync.dma_start(
                wk_sb[:], wk.rearrange("(dc p) c -> p dc c", p=128))
            for dc in range(4, 8):
                nc.sync.dma_start(xts[dc][:], xT[dc * 128:(dc + 1) * 128, :])
            nc.sync.dma_start(
                wv_sb[:], wv.rearrange("(dc p) c -> p dc c", p=128))

            # ---- constants ----
            bqk_sb = constp.tile([128, 2 * NPAIR], FP32)
            nc.sync.dma_start(bqk_sb[:], bqk[:])
            bvt_sb = constp.tile([128, HL * DH], FP32)
            nc.sync.dma_start(bvt_sb[:], bvt[:])
            tri_sb = constp.tile([128, 128], BF16)
            nc.sync.dma_start(tri_sb[:], trim[:])
            id_sb = constp.tile([128, 128], BF16)
            nc.sync.dma_start(id_sb[:], iden[:])
            wfa_sb = constp.tile([DH + 1, D], BF16)
            nc.sync.dma_start(wfa_sb[:], wfa[:])

            # ---- persistent activations ----
            qT = qkvp.tile([128, NPAIR, S], BF16)   # [(2 heads)*64e, pair, s]
            kT = qkvp.tile([128, NPAIR, S], BF16)
            vA = qkvp.tile([128, 8, HL, DH + 1], BF16)  # [t_in, t_chunk, head, e+1]
            zsum = qkvp.tile([DH, S], FP32)
            zsumB = qkvp.tile([DH, 512], FP32)  # sc=1 heads 4-7 partial sum
            rsA = qkvp.tile([DH, 256], FP32)    # received quarter, sc=0
            rsB1 = qkvp.tile([DH, 256], FP32)   # received quarter, sc=1 h0-3
            rsB2 = qkvp.tile([DH, 256], FP32)   # received quarter, sc=1 h4-7
            rs_aug = qkvp.tile([DH + 1, 512], BF16)
            nc.sync.dma_start(vA[:, :, :, DH:DH + 1], onesp[:])
            nc.sync.dma_start(rs_aug[DH:DH + 1, :], ones512[:])

            # ---- phase 1: projections (bf16 matmuls, fp32 psum) ----
            for p in range(NPAIR):
                psq = bigp.tile([128, 1024], FP32, tag="big")
                for s2 in range(2):
                    ssl = slice(s2 * 512, (s2 + 1) * 512)
                    for dc in range(8):
                        nc.tensor.matmul(
                            psq[:, ssl],
                            wq_sb[:, dc, p * 128:(p + 1) * 128],
                            xts[dc][:, ssl],
                            start=(dc == 0), stop=(dc == 7),
                        )
                nc.vector.tensor_scalar_add(
                    qT[:, p, :], psq[:], bqk_sb[:, p:p + 1])
                psk = bigp.tile([128, 1024], FP32, tag="big")
                for s2 in range(2):
                    ssl = slice(s2 * 512, (s2 + 1) * 512)
                    for dc in range(8):
                        nc.tensor.matmul(
                            psk[:, ssl],
                            wk_sb[:, dc, p * 128:(p + 1) * 128],
                            xts[dc][:, ssl],
                            start=(dc == 0), stop=(dc == 7),
                        )
                nc.vector.tensor_scalar_add(
                    kT[:, p, :], psk[:], bqk_sb[:, NPAIR + p:NPAIR + p + 1])

            for u in range(4):
                psv = bigp.tile([128, 1024], FP32, tag="big")
                for half in range(2):
                    t_c = 2 * u + half
                    for dc in range(8):
                        nc.tensor.matmul(
                            psv[:, half * 512:(half + 1) * 512],
                            xts[dc][:, t_c * 128:(t_c + 1) * 128],
                            wv_sb[:, dc, :],
                            start=(dc == 0), stop=(dc == 7),
                        )
                for half in range(2):
                    t_c = 2 * u + half
                    nc.vector.tensor_tensor(
                        vA[:, t_c, :, :DH],
                        psv[:, half * 512:(half + 1) * 512].rearrange(
                            "p (h e) -> p h e", h=HL),
                        bvt_sb[:].rearrange("p (h e) -> p h e", h=HL),
                        ALU.add,
                    )

            # ---- phase 2: attention, query-half (sc) outer for RS overlap ----
            def ff_block(j0, j1):
                for j in range(j0, j1):
                    for dcol in range(2):
                        dsl = slice(dcol * 512, (dcol + 1) * 512)
                        fps = zzp.tile([128, 512], FP32, tag="zz")
                        nc.tensor.matmul(
                            fps[:],
                            rs_aug[:, j * 128:(j + 1) * 128],
                            wfa_sb[:, dsl],
                            start=True, stop=True,
                        )
                        og = outp.tile([128, 512], FP32, tag="og")
                        nc.scalar.activation(og[:], fps[:], AF.Gelu)
                        nc.sync.dma_start(
                            out_ext[j * 128:(j + 1) * 128, dsl], og[:])

            def exchange(src_ap, rs_f32_tile):
                """Pairwise quarter ReduceScatter of a [64,512] fp32 half;
                rank g of the pair receives quarter g into rs_f32_tile."""
                zin = dramp.tile([2, DH, 256], FP32)
                zout = dramp.tile([DH, 256], FP32)
                for seg in range(2):
                    nc.sync.dma_start(
                        zin[seg], src_ap[:, seg * 256:(seg + 1) * 256])
                nc.gpsimd.collective_compute(
                    "ReduceScatter", ALU.add, replica_groups=RG,
                    ins=[zin[:].opt()], outs=[zout[:].opt()],
                )
                nc.sync.dma_start(rs_f32_tile[:], zout[:])

            for sc in range(2):
                C = 4 * sc + 4
                qsl = slice(sc * 512, (sc + 1) * 512)
                for h_loc in range(HL):
                    p, hh = h_loc // 2, h_loc % 2
                    rows = slice(hh * 64, hh * 64 + 64)
                    # sc=1 heads 4-7 accumulate separately so their exchange
                    # can start after the first four heads
                    acc = zsum[:, qsl] if (sc == 0 or h_loc < 4) else zsumB[:]
                    first = h_loc == 0 or (sc == 1 and h_loc == 4)
                    wT = wpool.tile([128, 8, 512], BF16, tag="wT")
                    for u in range(C // 2):
                        ps = bigp.tile([128, 1024], FP32, tag="big")
                        for half in range(2):
                            t_c = 2 * u + half
                            diag = t_c >= 4 * sc
                            zs = 128 * (t_c - 4 * sc) if diag else 0
                            off = half * 512 + zs
                            nc.tensor.matmul(
                                ps[:, off:(half + 1) * 512],
                                kT[rows, p, t_c * 128:(t_c + 1) * 128],
                                qT[rows, p, sc * 512 + zs:(sc + 1) * 512],
                                start=True, stop=not diag,
                                skip_group_check=True,
                            )
                            if diag:
                                nc.tensor.matmul(
                                    ps[:, off:off + 128],
                                    tri_sb[:],
                                    id_sb[:],
                                    start=False, stop=True,
                                    skip_group_check=True,
                                )
                        nc.scalar.activation(
                            wT[:, 2 * u:2 * u + 2, :], ps[:], AF.Exp, scale=0.125)
                    zaug = zzp.tile([128, 512], FP32, tag="zz")
                    for t_c in range(C):
                        zs = 128 * (t_c - 4 * sc) if t_c >= 4 * sc else 0
                        nc.tensor.matmul(
                            zaug[:DH + 1, zs:512],
                            vA[:, t_c, h_loc, :],
                            wT[:, t_c, zs:512],
                            start=(t_c == 0), stop=(t_c == C - 1),
                            skip_group_check=True,
                        )
                    # custom-DVE reciprocal requires base partition 0: copy
                    # the den row (psum partition 64) down to an SBUF tile.
                    dcopy = smallp.tile([1, 512], FP32, tag="dcopy")
                    nc.vector.tensor_copy(dcopy[:], zaug[DH:DH + 1, :])
                    recip = smallp.tile([1, 512], FP32, tag="recip")
                    nc.vector.reciprocal_approx_fast(recip[:], dcopy[:])
                    bcast = smallp.tile([DH, 512], FP32, tag="bcast")
                    nc.gpsimd.partition_broadcast(bcast[:], recip[:])
                    if first:
                        nc.vector.tensor_tensor(
                            acc, zaug[:DH, :], bcast[:], ALU.mult)
                    else:
                        tmp = smallp.tile([DH, 512], FP32, tag="ztmp")
                        nc.vector.tensor_tensor(
                            tmp[:], zaug[:DH, :], bcast[:], ALU.mult)
                        nc.vector.tensor_tensor(acc, acc, tmp[:], ALU.add)

                    if sc == 1 and h_loc == 3:
                        # exchange heads 0-3 of the second half early
                        exchange(zsum[:, qsl], rsB1)
                    if sc == 1 and h_loc == 5:
                        # first-quarter ff hidden under the sc=1 head loop,
                        # once the sc=0 exchange has certainly landed
                        nc.vector.tensor_copy(rs_aug[:DH, 0:256], rsA[:])
                        ff_block(0, 2)

                if sc == 0:
                    exchange(zsum[:, qsl], rsA)

            # heads 4-7 of the second half: final, smallest exposed exchange
            exchange(zsumB[:], rsB2)
            nc.vector.tensor_tensor(rsB1[:], rsB1[:], rsB2[:], ALU.add)
            nc.vector.tensor_copy(rs_aug[:DH, 256:512], rsB1[:])

            # ---- phase 3: remaining ff quarter (queries from sc=1) ----
            ff_block(2, 4)

    nc.compile()
    return nc


_NC = None


def _get_nc():
    global _NC
    if _NC is None:
        _NC = build_nc()
    return _NC


def make_in_maps(x, Wq, bq, Wk, bk, Wv, bv, Wf, bf):
    import ml_dtypes

    bf16 = ml_dtypes.bfloat16
    x, Wq, bq, Wk, bk, Wv, bv, Wf, bf = (
        np.asarray(a, dtype=np.float32)
        for a in (x, Wq, bq, Wk, bk, Wv, bv, Wf, bf))

    ii, jj = np.meshgrid(np.arange(128), np.arange(128), indexing="ij")
    # stationary triangle for the mask matmul: rows k (contraction), cols m;
    # NEG where m > k so that (tri^T @ I)[m, j] = NEG * (m > j)
    trim = np.where(jj > ii, np.float32(NEG), 0.0).astype(bf16)
    iden = np.eye(128, dtype=np.float32).astype(bf16)
    wfa = np.concatenate([Wf, bf.reshape(1, D)], axis=0).astype(bf16)

    in_maps = []
    for c in range(8):
        b, g = c // 2, c % 2
        hs = slice(g * HL, (g + 1) * HL)
        bqk_l = np.empty((128, 2 * NPAIR), np.float32)
        for p in range(NPAIR):
            bqk_l[:, p] = bq[g * HL + 2 * p: g * HL + 2 * p + 2].reshape(128)
            bqk_l[:, NPAIR + p] = bk[g * HL + 2 * p: g * HL + 2 * p + 2].reshape(128)
        in_maps.append({
            "xT": np.ascontiguousarray(x[b].T).astype(bf16),
            "wq": np.ascontiguousarray(
                Wq[hs].transpose(1, 0, 2).reshape(D, HL * DH)).astype(bf16),
            "wk": np.ascontiguousarray(
                Wk[hs].transpose(1, 0, 2).reshape(D, HL * DH)).astype(bf16),
            "wv": np.ascontiguousarray(
                Wv[hs].transpose(1, 0, 2).reshape(D, HL * DH)).astype(bf16),
            "bqk": bqk_l,
            "bvt": np.ascontiguousarray(
                np.broadcast_to(bv[hs].reshape(1, HL * DH), (128, HL * DH))),
            "wfa": wfa,
            "trim": trim,
            "iden": iden,
            "onesp": np.ones((128, 8, HL), bf16),
            "ones512": np.ones((1, 512), bf16),
        })
    return in_maps


def run(in_maps, trace=False, **kw):
    nc = _get_nc()
    return run_bass_kernel_spmd(nc, in_maps, list(range(8)), trace=trace, **kw)


def assemble(results):
    """Reassemble per-core [512, D] outputs into [B, S, D].

    Core c = (b, g): rows 0-255 are queries [256g, 256g+256), rows 256-511
    are queries [512+256g, 512+256g+256) of batch b.
    """
    out = np.empty((B, S, D), np.float32)
    for c in range(8):
        b, g = c // 2, c % 2
        r = results[c]["out"]
        out[b, 256 * g:256 * (g + 1), :] = r[0:256]
        out[b, 512 + 256 * g:512 + 256 * (g + 1), :] = r[256:512]
    return out


def kernel(x, Wq, bq, Wk, bk, Wv, bv, Wf, bf):
    in_maps = make_in_maps(x, Wq, bq, Wk, bk, Wv, bv, Wf, bf)
    res = run(in_maps)
    return assemble(res.results)


if __name__ == "__main__":
    nc = build_nc()
    print("build OK")
